# revision 5
# baseline (speedup 1.0000x reference)
"""SAGAN self-attention block on 8 TRN2 NeuronCores.

Reference (per batch element b, N = H*W = 4096, C = 512, D = 64):
    f = x @ Wf + bf ; g = x @ Wg + bg ; h = x @ Wh + bh      # [N, D]
    s = f @ g.T                                              # [N, N]
    attn = softmax(s, axis=-1)
    ctx = attn @ h                                           # [N, D]
    o = (gamma * ctx) @ Wv + bv + x                          # [N, C]

Sharding: data-parallel over batch B=8 -> one batch element per core, no
collectives. Weights replicated.

Device algorithm (per core), all matmuls bf16 with f32 PSUM accumulation:
  - load x [4096, 512] f32; cast to bf16; PE-transpose to xT [512, 4096]
  - fT = Wf.T @ xT (+bf), gT likewise -> [64, 4096] bf16 ("d on partitions")
  - h_aug[m, :] = [x@Wh + bh, 1.0]  -> [4096, 65] bf16 (m on partitions)
  - unnormalized softmax (no max subtraction: |s| <~ 50 so exp fits f32/bf16):
      for each n-group of 1024 columns:
        for each m-tile of 128 rows:
          S'[m, n] = g_m . f_n  (matmul, K=64)   -> PSUM [128, 1024]
          E' = exp(S')  (ScalarE)                -> SBUF bf16
          ctxT[0:65, n] += h_aug[m].T @ E'       -> PSUM [65, 1024]
        row 64 of ctxT = sum_m E' = softmax denominator (ones column trick)
  - out[n, :] = (ctxT[:, n].T @ [gamma*Wv ; bv]) * (1/denom[n]) + x[n, :]
      (bv rides on the denom row so it survives the 1/denom scaling)
"""

import numpy as np
import ml_dtypes

BF16 = ml_dtypes.bfloat16

B, HH, WW, C = 8, 64, 64, 512
D = C // 8          # 64
N_FULL = HH * WW    # 4096
P = 128
CC = C // P         # 4  (c-chunks of 128)

_CACHE: dict = {}


def _build(n: int):
    """Build + compile the single-core Bass program (same NEFF on all 8 cores)."""
    import concourse.mybir as mybir
    from concourse import bacc
    from concourse.tile import TileContext

    f32 = mybir.dt.float32
    bf16 = mybir.dt.bfloat16
    ADD = mybir.AluOpType.add
    MULT = mybir.AluOpType.mult
    EXP = mybir.ActivationFunctionType.Exp

    n_tiles = n // P
    jw = min(1024, n)          # attention n-group width (2 PSUM banks)
    jj_n = n // jw
    h2_n = jw // 512

    nc = bacc.Bacc("TRN2", target_bir_lowering=False, debug=False)

    x_d = nc.dram_tensor("x", [n, C], f32, kind="ExternalInput")
    wf_d = nc.dram_tensor("wf", [C, D], bf16, kind="ExternalInput")
    wg_d = nc.dram_tensor("wg", [C, D], bf16, kind="ExternalInput")
    wh_d = nc.dram_tensor("wh", [C, D], bf16, kind="ExternalInput")
    bf_d = nc.dram_tensor("bfp", [D, 1], f32, kind="ExternalInput")
    bg_d = nc.dram_tensor("bgp", [D, 1], f32, kind="ExternalInput")
    bh_d = nc.dram_tensor("bhp", [1, D], bf16, kind="ExternalInput")
    on_d = nc.dram_tensor("onesp", [1, P], bf16, kind="ExternalInput")
    wv_d = nc.dram_tensor("wv", [D + 1, C], bf16, kind="ExternalInput")
    id_d = nc.dram_tensor("ident", [P, P], bf16, kind="ExternalInput")
    out_d = nc.dram_tensor("out", [n, C], f32, kind="ExternalOutput")

    x_t = x_d.rearrange("(i p) c -> i p c", p=P)
    o_t = out_d.rearrange("(i p) c -> i p c", p=P)

    with TileContext(nc) as tc:
        with (
            tc.tile_pool(name="const", bufs=1) as cpool,
            tc.tile_pool(name="big", bufs=1) as bigpool,
            tc.tile_pool(name="xb", bufs=3) as xpool,
            tc.tile_pool(name="ep", bufs=3) as epool,
            tc.tile_pool(name="ct", bufs=2) as ctpool,
            tc.tile_pool(name="os", bufs=3) as opool,
            tc.tile_pool(name="sm", bufs=4) as smpool,
            tc.tile_pool(name="psA", bufs=2, space="PSUM") as psA,
            tc.tile_pool(name="psB", bufs=1, space="PSUM") as psB,
            tc.tile_pool(name="psC", bufs=2, space="PSUM") as psC,
        ):
            # ---- replicated constants -> SBUF
            wf_sb = cpool.tile([P, CC, D], bf16)
            nc.sync.dma_start(wf_sb, wf_d.rearrange("(cc p) d -> p cc d", p=P))
            wg_sb = cpool.tile([P, CC, D], bf16)
            nc.sync.dma_start(wg_sb, wg_d.rearrange("(cc p) d -> p cc d", p=P))
            wh_sb = cpool.tile([P, CC, D], bf16)
            nc.sync.dma_start(wh_sb, wh_d.rearrange("(cc p) d -> p cc d", p=P))
            bf_sb = cpool.tile([D, 1], f32)
            nc.sync.dma_start(bf_sb, bf_d[:, :])
            bg_sb = cpool.tile([D, 1], f32)
            nc.sync.dma_start(bg_sb, bg_d[:, :])
            bh_sb = cpool.tile([1, D], bf16)
            nc.sync.dma_start(bh_sb, bh_d[:, :])
            ones_sb = cpool.tile([1, P], bf16)
            nc.sync.dma_start(ones_sb, on_d[:, :])
            wv_sb = cpool.tile([D + 1, C], bf16)
            nc.sync.dma_start(wv_sb, wv_d[:, :])
            id_sb = cpool.tile([P, P], bf16)
            nc.sync.dma_start(id_sb, id_d[:, :])

            # ---- persistent SBUF tensors
            xres = bigpool.tile([P, n_tiles, C], f32)    # x rows (residual + cast src)
            xT = bigpool.tile([P, CC, n], bf16)          # x transposed (c on partitions)
            FT = bigpool.tile([D, n], bf16)              # f.T
            GT = bigpool.tile([D, n], bf16)              # g.T
            haug = bigpool.tile([P, n_tiles, D + 1], bf16)
            nc.gpsimd.memset(haug[:, :, D:D + 1], 1.0)

            # ---- prologue: load x, cast to bf16, transpose via PE
            for i in range(n_tiles):
                nc.sync.dma_start(xres[:, i, :], x_t[i])
                xb = xpool.tile([P, C], bf16, tag="xb")
                nc.vector.tensor_copy(out=xb, in_=xres[:, i, :])
                tp = psA.tile([P, C], bf16, tag="sp")
                for cc in range(CC):
                    nc.tensor.transpose(
                        tp[:, cc * P:(cc + 1) * P], xb[:, cc * P:(cc + 1) * P], id_sb
                    )
                nc.vector.tensor_copy(
                    out=xT[:, :, i * P:(i + 1) * P],
                    in_=tp.rearrange("p (cc q) -> p cc q", q=P),
                )

            # ---- projections: fT/gT ([64, n], d on partitions), h_aug rows
            for jc in range(n // 512):
                sl = slice(jc * 512, (jc + 1) * 512)
                fps = psA.tile([D, 512], f32, tag="sp")
                for cc in range(CC):
                    nc.tensor.matmul(
                        fps, lhsT=wf_sb[:, cc, :], rhs=xT[:, cc, sl],
                        start=(cc == 0), stop=(cc == CC - 1),
                    )
                nc.vector.tensor_scalar(FT[:, sl], fps, bf_sb, None, ADD)
                gps = psA.tile([D, 512], f32, tag="sp")
                for cc in range(CC):
                    nc.tensor.matmul(
                        gps, lhsT=wg_sb[:, cc, :], rhs=xT[:, cc, sl],
                        start=(cc == 0), stop=(cc == CC - 1),
                    )
                nc.vector.tensor_scalar(GT[:, sl], gps, bg_sb, None, ADD)

            for i in range(n_tiles):
                hps = psA.tile([P, D], f32, tag="sp")
                for cc in range(CC):
                    nc.tensor.matmul(
                        hps, lhsT=xT[:, cc, i * P:(i + 1) * P], rhs=wh_sb[:, cc, :],
                        start=(cc == 0), stop=False,
                    )
                # += ones.T @ bh  (broadcasts the bias over the m rows)
                nc.tensor.matmul(hps, lhsT=ones_sb, rhs=bh_sb, start=False, stop=True)
                nc.vector.tensor_copy(out=haug[:, i, 0:D], in_=hps)

            # ---- attention main loop
            for jj in range(jj_n):
                ctx = psB.tile([D + 1, jw], f32, tag="ctx")
                for i in range(n_tiles):
                    sp = psA.tile([P, jw], f32, tag="sp")
                    for h2 in range(h2_n):
                        nc.tensor.matmul(
                            sp[:, h2 * 512:(h2 + 1) * 512],
                            lhsT=GT[:, i * P:(i + 1) * P],
                            rhs=FT[:, jj * jw + h2 * 512: jj * jw + (h2 + 1) * 512],
                            start=True, stop=True,
                        )
                    ep = epool.tile([P, jw], bf16, tag="ep")
                    nc.scalar.activation(ep, sp, EXP)
                    for h2 in range(h2_n):
                        nc.tensor.matmul(
                            ctx[:, h2 * 512:(h2 + 1) * 512],
                            lhsT=haug[:, i, :],
                            rhs=ep[:, h2 * 512:(h2 + 1) * 512],
                            start=(i == 0), stop=(i == n_tiles - 1),
                        )

                # ---- epilogue for this n-group
                ct = ctpool.tile([D + 1, jw], bf16, tag="ct")
                nc.vector.tensor_copy(out=ct, in_=ctx)
                for t in range(jw // P):
                    it = jj * (jw // P) + t
                    tsl = slice(t * P, (t + 1) * P)
                    dt = psC.tile([P, 1], bf16, tag="oc")
                    nc.tensor.transpose(dt, ct[D:D + 1, tsl], id_sb[D:D + 1, D:D + 1])
                    rc = smpool.tile([P, 1], f32, tag="rc")
                    nc.vector.reciprocal(rc, dt)
                    op = psC.tile([P, C], f32, tag="oc")
                    nc.tensor.matmul(op, lhsT=ct[:, tsl], rhs=wv_sb, start=True, stop=True)
                    osb = opool.tile([P, C], f32, tag="os")
                    nc.vector.tensor_scalar(osb, op, rc, None, MULT)
                    nc.vector.tensor_tensor(osb, osb, xres[:, it, :], ADD)
                    nc.sync.dma_start(o_t[it], osb)

    nc.compile()
    return nc


def get_program(n: int = N_FULL):
    if n not in _CACHE:
        _CACHE[n] = _build(n)
    return _CACHE[n]


def make_weight_maps(Wf, bf, Wg, bg, Wh, bh, Wv, bv, gamma):
    """Host-side layout prep of the tiny replicated weights."""
    wv_aug = np.concatenate(
        [np.float32(gamma) * np.asarray(Wv, np.float32),
         np.asarray(bv, np.float32)[None, :]], axis=0)
    return {
        "wf": np.ascontiguousarray(np.asarray(Wf, np.float32).astype(BF16)),
        "wg": np.ascontiguousarray(np.asarray(Wg, np.float32).astype(BF16)),
        "wh": np.ascontiguousarray(np.asarray(Wh, np.float32).astype(BF16)),
        "bfp": np.ascontiguousarray(np.asarray(bf, np.float32).reshape(D, 1)),
        "bgp": np.ascontiguousarray(np.asarray(bg, np.float32).reshape(D, 1)),
        "bhp": np.ascontiguousarray(np.asarray(bh, np.float32).astype(BF16).reshape(1, D)),
        "onesp": np.ones((1, P), dtype=BF16),
        "wv": np.ascontiguousarray(wv_aug.astype(BF16)),
        "ident": np.ascontiguousarray(np.eye(P, dtype=BF16)),
    }


def kernel(x, Wf, bf, Wg, bg, Wh, bh, Wv, bv, gamma):
    from concourse.bass_utils import run_bass_kernel_spmd

    x = np.asarray(x, np.float32)
    b, hh, ww, c = x.shape
    n = hh * ww
    assert (b, c) == (B, C)

    nc = get_program(n)
    base = make_weight_maps(Wf, bf, Wg, bg, Wh, bh, Wv, bv, gamma)
    xf = x.reshape(b, n, c)
    in_maps = [dict(base, x=np.ascontiguousarray(xf[i])) for i in range(b)]

    res = run_bass_kernel_spmd(nc, in_maps, core_ids=list(range(b)))
    out = np.stack([res.results[i]["out"] for i in range(b)], axis=0)
    return np.ascontiguousarray(out.reshape(b, hh, ww, c).astype(np.float32))
